# revision 3
# baseline (speedup 1.0000x reference)
"""Trainium2 Bass kernel for nn_MessageUpdatePore (gnn_message_passing).

Algebraic collapse: with idx2_oh == one_hot(idx2) and perms1 == perms2,
the permutation-equivariant module reduces to per-edge dense algebra
    z    = A1[b,idx1[e]] + A2[b,idx2[e]] + b_eq + bonds[b,e] @ W3
    lat  = leaky_relu(z);  lat *= sigmoid(lat @ W_att + b_att)
    out[b, idx2[e]] += lat
where A1 = sites1 @ W[:CIN], A2 = sites2 @ W[CIN:2CIN] fold host-side
(O(nodes)), W = mean_g W_eq.

Structure (driven by HW NTFF traces; E sharded 256 edges/core over 8
cores, [K,B*O] partials summed on host):
  * A1/A2 gathers fold host-side into one per-edge table Ag, removing the
    [96,E] one-hot matmuls and the software-DGE gpsimd ring whose slow
    transfers gated PE start by ~2us.
  * Everything device-side is bf16 (one-hots exact; tables ~0.4% rel err
    vs the 2e-2 gate): halves DMA bytes, doubles PE rate.
  * Both batches share each z matmul via a block-diagonal W3 on the
    contraction dim: one K=64 matmul per 128-edge chunk.
  * leaky_relu runs on the otherwise-idle Activation engine as Prelu
    (alpha=0.01).  get_activation_tables is filtered so Prelu resolves to
    the 'sigmoid_and_others' act-table set: the ACT_TABLE_LOADs all hoist
    off the critical path instead of a 1.3us reload before the sigmoid.
  * attention dot via scalar_tensor_tensor with accum_out (one DVE op per
    (chunk,batch)), per-batch [128,1] sigmoid, per-batch scale -- the
    b0 path races ahead while b1 is still reducing.
  * 2 input DMAs on the two hardware-DGE rings (sync + scalar); output
    split per batch into two DMAs (sync + scalar) fed by per-batch PSUM
    scatter accumulators, copies on DVE and Act.

Known-bad variants (all verified on HW): accumulating Ag into PSUM via an
identity matmul faults the PE exec unit (NRT_EXEC_UNIT_UNRECOVERABLE);
InstTensorTensorReduce faults the device; stride-0 broadcast scales work
but are not faster; moving scales to GpSimd or batching sigmoids per
chunk regresses ~1.5us.
"""

from contextlib import ExitStack

import numpy as np
import ml_dtypes

import concourse.bacc as bacc
import concourse.mybir as mybir
import concourse.tile as tile
from concourse.bass_utils import run_bass_kernel_spmd

B, E, N1, K, CIN, CB, COUT, G = 2, 2048, 96, 32, 64, 32, 64, 4
F = 2 * CIN + CB           # 160
NCORES = 8
ES = E // NCORES           # 256 edges per core
ECH = ES // 128            # 2 edge chunks of 128
NEG_SLOPE = 0.01
f32 = mybir.dt.float32
bf16 = mybir.dt.bfloat16
NO = B * COUT              # 128: z columns, (b, o) pairs

# dA [64, xA]: bonds (2-batch stacked on contraction) + block-diag W3
A_BONDS = 0                # ECH chunks of [64, 128]
A_W3BD = ECH * 128         # [64, NO]
XA = A_W3BD + NO
# dB [128, xB]: per-chunk Ag + W_att row + one-hot(idx2) + b_att
B_AG = 0                   # ECH chunks of [128, NO]
B_WATT = B_AG + ECH * NO   # [128, NO]
B_OH2 = B_WATT + NO        # ECH chunks of [128, K]
B_BATT = B_OH2 + ECH * K   # [128, 1]
XB = B_BATT + 1

# toggles for A/B probes (env-overridable for bisects)
import os as _os
LEAKY_ON_ACT = _os.environ.get("KV2_LEAKY_ACT", "1") == "1"
TTR_ATT = _os.environ.get("KV2_TTR", "0") == "1"  # InstTensorTensorReduce faults HW
BCAST_SCALE = _os.environ.get("KV2_BCAST", "0") == "1"  # works but not faster
ACT_TABLE_PATCH = _os.environ.get("KV2_ACTPATCH", "1") == "1"

_programs: dict = {}


def _patch_act_tables():
    """Make Prelu resolve to the same act-table set as Sigmoid so the
    compiler emits a single hoisted ACT_TABLE_LOAD.  Set positions (the
    act_func_set_id namespace) are preserved; only membership shrinks."""
    from concourse.hw_specs import get_activation_tables as _orig

    T = mybir.ActivationFunctionType

    def patched(arch):
        tabs = {k: set(v) for k, v in _orig(arch).items()}
        shared = tabs.get("sigmoid_and_others")
        if not shared or T.Prelu not in shared or T.Sigmoid not in shared:
            return tabs
        for name, fns in tabs.items():
            if name != "sigmoid_and_others":
                fns.discard(T.Prelu)
                fns.discard(T.Sigmoid)
        return tabs

    bacc.get_activation_tables = patched


if ACT_TABLE_PATCH:
    _patch_act_tables()


def _build_program(use_batt: bool):
    nc = bacc.Bacc(
        "TRN2", target_bir_lowering=False, debug=False, num_devices=NCORES
    )
    dA = nc.dram_tensor("dA", [64, XA], bf16, kind="ExternalInput")
    dB = nc.dram_tensor("dB", [128, XB], bf16, kind="ExternalInput")
    out_d = nc.dram_tensor("out", [K, NO], f32, kind="ExternalOutput")
    mult, add = mybir.AluOpType.mult, mybir.AluOpType.add

    with tile.TileContext(nc) as tc, ExitStack() as ctx:
        const = ctx.enter_context(tc.tile_pool(name="const", bufs=1))
        work = ctx.enter_context(tc.tile_pool(name="work", bufs=2))
        ps_z = ctx.enter_context(tc.tile_pool(name="ps_z", bufs=2, space="PSUM"))
        ps_o = ctx.enter_context(tc.tile_pool(name="ps_o", bufs=1, space="PSUM"))

        tA = const.tile([64, XA], bf16, tag="tA", name="tA")
        nc.sync.dma_start(tA[:], dA[:])
        tB = const.tile([128, XB], bf16, tag="tB", name="tB")
        nc.scalar.dma_start(tB[:], dB[:])

        w3bd = tA[:, A_W3BD : A_W3BD + NO]
        wattc = tB[:, B_WATT : B_WATT + NO]

        zs = []
        for c in range(ECH):
            z = ps_z.tile([128, NO], f32, tag="z", name=f"z{c}")
            nc.tensor.matmul(
                z[:], tA[:, A_BONDS + c * 128 : A_BONDS + (c + 1) * 128], w3bd,
                start=True, stop=True,
            )
            zs.append(z)

        if use_batt:
            battf = work.tile([128, 1], f32, tag="battf", name="battf")
            nc.vector.tensor_copy(battf[:], tB[:, B_BATT : B_BATT + 1])

        lats = []
        for c in range(ECH):
            agc = tB[:, B_AG + c * NO : B_AG + (c + 1) * NO]
            t1 = work.tile([128, NO], bf16, tag="t1", name=f"t1_{c}")
            nc.vector.tensor_add(t1[:], zs[c][:], agc)
            pre = t1[:]
            lat = const.tile([128, NO], bf16, tag=f"lat{c}", name=f"lat{c}")
            if LEAKY_ON_ACT:
                nc.scalar.activation(
                    lat[:], pre, mybir.ActivationFunctionType.Prelu,
                    alpha=NEG_SLOPE,
                )
            else:
                tmp = work.tile([128, NO], bf16, tag="tmp", name=f"tmp{c}")
                nc.vector.tensor_scalar_mul(tmp[:], pre, NEG_SLOPE)
                nc.vector.tensor_max(lat[:], tmp[:], pre)
            lats.append(lat)

            junk = work.tile([128, NO], bf16, tag="junk", name=f"junk{c}")
            for b in range(B):
                scol = work.tile([128, 1], f32, tag=f"scol{b}", name=f"scol{c}_{b}")
                nc.vector.scalar_tensor_tensor(
                    out=junk[:, b * COUT : (b + 1) * COUT],
                    in0=lat[:, b * COUT : (b + 1) * COUT], scalar=1.0,
                    in1=wattc[:, b * COUT : (b + 1) * COUT],
                    op0=mult, op1=mult, accum_out=scol[:],
                )
                att1 = work.tile([128, 1], f32, tag=f"att{b}", name=f"att{c}_{b}")
                nc.scalar.activation(
                    att1[:], scol[:], mybir.ActivationFunctionType.Sigmoid,
                    bias=battf[:, 0:1] if use_batt else 0.0,
                )
                sl = lat[:, b * COUT : (b + 1) * COUT]
                nc.vector.tensor_scalar_mul(sl, sl, att1[:])

        # per-batch scatter accumulators -> two copies and two output DMAs
        # on the two HW rings, so the out tail overlaps instead of chaining
        o_bs = [ps_o.tile([K, COUT], f32, tag=f"ob{b}", name=f"ob{b}") for b in range(B)]
        for c in range(ECH):
            oh2c = tB[:, B_OH2 + c * K : B_OH2 + (c + 1) * K]
            for b in range(B):
                nc.tensor.matmul(
                    o_bs[b][:], oh2c,
                    lats[c][:, b * COUT : (b + 1) * COUT],
                    start=(c == 0), stop=(c == ECH - 1),
                )
        o_sb0 = work.tile([K, COUT], f32, tag="osb0", name="osb0")
        nc.vector.tensor_copy(o_sb0[:], o_bs[0][:])
        nc.sync.dma_start(out_d[:, 0:COUT], o_sb0[:])
        o_sb1 = work.tile([K, COUT], f32, tag="osb1", name="osb1")
        nc.scalar.activation(o_sb1[:], o_bs[1][:], mybir.ActivationFunctionType.Copy)
        nc.scalar.dma_start(out_d[:, COUT:NO], o_sb1[:])

    nc.compile()
    return nc


def _get_program(use_batt: bool):
    if use_batt not in _programs:
        _programs[use_batt] = _build_program(use_batt)
    return _programs[use_batt]


def _prepare(inputs):
    """Host fold: group-mean weights, node tables through W, per-edge Ag."""
    sites1 = np.asarray(inputs["sites1"], np.float32)
    sites2 = np.asarray(inputs["sites2"], np.float32)
    bonds = np.asarray(inputs["bonds"], np.float32)
    W_eq = np.asarray(inputs["W_eq"], np.float32)
    b_eq = np.asarray(inputs["b_eq"], np.float32)
    W_att = np.asarray(inputs["W_att"], np.float32)
    b_att = np.asarray(inputs["b_att"], np.float32)
    idx1 = np.asarray(inputs["idx1"])
    idx2 = np.asarray(inputs["idx2"])

    W_eff = W_eq.mean(axis=0)                       # [F, COUT]
    A1 = sites1 @ W_eff[0:CIN]                      # [B, N1, COUT]
    A2 = sites2 @ W_eff[CIN : 2 * CIN]              # [B, K, COUT]
    W3 = W_eff[2 * CIN : F]                         # [CB, COUT]
    Ag = A1[:, idx1] + A2[:, idx2] + b_eq[None, None, :]   # [B, E, COUT]

    w3bd = np.zeros((64, NO), np.float32)
    w3bd[0:CB, 0:COUT] = W3
    w3bd[CB:64, COUT:NO] = W3

    oh2 = (idx2[:, None] == np.arange(K)[None, :]).astype(np.float32)  # [E, K]
    use_batt = bool(np.any(b_att != 0.0))

    in_maps = []
    for m in range(NCORES):
        dA = np.zeros((64, XA), np.float32)
        dB = np.zeros((128, XB), np.float32)
        for c in range(ECH):
            rows = slice(m * ES + c * 128, m * ES + (c + 1) * 128)
            for b in range(B):
                dA[b * CB : (b + 1) * CB, A_BONDS + c * 128 : A_BONDS + (c + 1) * 128] = (
                    bonds[b, rows].T
                )
                dB[:, B_AG + c * NO + b * COUT : B_AG + c * NO + (b + 1) * COUT] = (
                    Ag[b, rows]
                )
            dB[:, B_OH2 + c * K : B_OH2 + (c + 1) * K] = oh2[rows]
        dA[:, A_W3BD : A_W3BD + NO] = w3bd
        for b in range(B):
            dB[:, B_WATT + b * COUT : B_WATT + (b + 1) * COUT] = W_att[:, 0][None, :]
        dB[:, B_BATT] = b_att[0]
        in_maps.append({
            "dA": dA.astype(ml_dtypes.bfloat16),
            "dB": dB.astype(ml_dtypes.bfloat16),
        })
    return use_batt, in_maps


def _numpy_fallback(inputs):
    """Exact reference semantics (pathological inputs only)."""
    sites1 = np.asarray(inputs["sites1"], np.float32)
    sites2 = np.asarray(inputs["sites2"], np.float32)
    bonds = np.asarray(inputs["bonds"], np.float32)
    W_eq = np.asarray(inputs["W_eq"], np.float32)
    b_eq = np.asarray(inputs["b_eq"], np.float32)
    W_att = np.asarray(inputs["W_att"], np.float32)
    b_att = np.asarray(inputs["b_att"], np.float32)
    idx2_oh = np.asarray(inputs["idx2_oh"], np.float32)
    idx1 = np.asarray(inputs["idx1"])
    idx2 = np.asarray(inputs["idx2"])
    perms1 = np.asarray(inputs["perms1"])
    perms2 = np.asarray(inputs["perms2"])
    Gn, Kn = perms1.shape
    inv2 = np.argsort(perms2, axis=1)
    out = np.zeros((B, Kn, COUT), np.float32)
    for b in range(B):
        vec = np.concatenate([sites1[b][idx1], sites2[b][idx2], bonds[b]], axis=1)
        zg = np.stack([vec @ W_eq[g] for g in range(Gn)])        # [G, E, O]
        y = np.zeros((E, COUT, Kn), np.float32)
        for g in range(Gn):
            sel = idx2_oh[:, perms1[g][inv2[g]]]                 # [E, K]
            y += zg[g][:, :, None] * sel[:, None, :]
        y /= Gn
        y = y + b_eq[None, :, None]
        y = np.maximum(y, NEG_SLOPE * y)
        lat = np.einsum("eok,ek->eo", y, idx2_oh)
        att = 1.0 / (1.0 + np.exp(-(lat @ W_att[:, 0] + b_att[0])))
        lat = att[:, None] * lat
        np.add.at(out[b], idx2, lat)
    return out


def _run(inputs, trace=False, **run_kwargs):
    idx2 = np.asarray(inputs["idx2"])
    idx2_oh = np.asarray(inputs["idx2_oh"], np.float32)
    expected_oh = (idx2[:, None] == np.arange(K)[None, :]).astype(np.float32)
    perms1 = np.asarray(inputs["perms1"])
    perms2 = np.asarray(inputs["perms2"])
    inv2 = np.argsort(perms2, axis=1)
    c = np.take_along_axis(perms1, inv2, axis=1) == np.arange(K)[None, :]
    if not (np.array_equal(idx2_oh, expected_oh) and c.all()):
        return _numpy_fallback(inputs), None

    use_batt, in_maps = _prepare(inputs)
    nc = _get_program(use_batt)
    res = None
    last_err = None
    for _attempt in range(3):
        try:
            res = run_bass_kernel_spmd(
                nc, in_maps, list(range(NCORES)), trace=trace, **run_kwargs
            )
            break
        except Exception as e:  # transient device/tunnel flakes
            last_err = e
    if res is None:
        raise last_err
    acc = np.zeros((K, NO), np.float32)
    for r in res.results:
        acc += r["out"]
    out = acc.reshape(K, B, COUT).transpose(1, 0, 2)
    return np.ascontiguousarray(out), res


def kernel(**inputs) -> np.ndarray:
    out, _ = _run(inputs)
    return out


# revision 4
# speedup vs baseline: 1.0262x; 1.0262x over previous
"""Trainium2 Bass kernel for nn_MessageUpdatePore (gnn_message_passing).

Algebraic collapse: with idx2_oh == one_hot(idx2) and perms1 == perms2,
the permutation-equivariant module reduces to per-edge dense algebra
    z    = A1[b,idx1[e]] + A2[b,idx2[e]] + b_eq + bonds[b,e] @ W3
    lat  = leaky_relu(z);  lat *= sigmoid(lat @ W_att + b_att)
    out[b, idx2[e]] += lat
where A1 = sites1 @ W[:CIN], A2 = sites2 @ W[CIN:2CIN] fold host-side
(O(nodes)), W = mean_g W_eq.

Structure (driven by HW NTFF traces; E sharded 256 edges/core over 8
cores, [K,B*O] partials summed on host):
  * A1/A2 gathers fold host-side into one per-edge table Ag, removing the
    [96,E] one-hot matmuls and the software-DGE gpsimd ring whose slow
    transfers gated PE start by ~2us.
  * Everything device-side is bf16 (one-hots exact; tables ~0.4% rel err
    vs the 2e-2 gate): halves DMA bytes, doubles PE rate.
  * Both batches share each z matmul via a block-diagonal W3 on the
    contraction dim: one K=64 matmul per 128-edge chunk.
  * leaky_relu runs on the otherwise-idle Activation engine as Prelu
    (alpha=0.01).  get_activation_tables is filtered so Prelu resolves to
    the 'sigmoid_and_others' act-table set: the ACT_TABLE_LOADs all hoist
    off the critical path instead of a 1.3us reload before the sigmoid.
  * attention dot via scalar_tensor_tensor with accum_out (one DVE op per
    (chunk,batch)), per-batch [128,1] sigmoid, per-batch scale -- the
    b0 path races ahead while b1 is still reducing.
  * 2 input DMAs on the two hardware-DGE rings (sync + scalar); output
    split per batch into two DMAs (sync + scalar) fed by per-batch PSUM
    scatter accumulators, copies on DVE and Act.

Known-bad variants (all verified on HW): accumulating Ag into PSUM via an
identity matmul faults the PE exec unit (NRT_EXEC_UNIT_UNRECOVERABLE);
InstTensorTensorReduce faults the device; stride-0 broadcast scales work
but are not faster; moving scales to GpSimd or batching sigmoids per
chunk regresses ~1.5us.
"""

from contextlib import ExitStack

import numpy as np
import ml_dtypes

import concourse.bacc as bacc
import concourse.mybir as mybir
import concourse.tile as tile
from concourse.bass_utils import run_bass_kernel_spmd

B, E, N1, K, CIN, CB, COUT, G = 2, 2048, 96, 32, 64, 32, 64, 4
F = 2 * CIN + CB           # 160
NCORES = 8
ES = E // NCORES           # 256 edges per core
ECH = ES // 128            # 2 edge chunks of 128
NEG_SLOPE = 0.01
f32 = mybir.dt.float32
bf16 = mybir.dt.bfloat16
NO = B * COUT              # 128: z columns, (b, o) pairs

# dA [64, xA]: bonds (2-batch stacked on contraction) + block-diag W3
A_BONDS = 0                # ECH chunks of [64, 128]
A_W3BD = ECH * 128         # [64, NO]
XA = A_W3BD + NO
# dB [128, xB]: per-chunk Ag + W_att row + one-hot(idx2) + b_att
B_AG = 0                   # ECH chunks of [128, NO]
B_WATT = B_AG + ECH * NO   # [128, NO]
B_OH2 = B_WATT + NO        # ECH chunks of [128, K]
B_BATT = B_OH2 + ECH * K   # [128, 1]
XB = B_BATT + 1

# toggles for A/B probes (env-overridable for bisects)
import os as _os
LEAKY_ON_ACT = _os.environ.get("KV2_LEAKY_ACT", "1") == "1"
TTR_ATT = _os.environ.get("KV2_TTR", "0") == "1"  # InstTensorTensorReduce faults HW
BCAST_SCALE = _os.environ.get("KV2_BCAST", "0") == "1"  # works but not faster
ACT_TABLE_PATCH = _os.environ.get("KV2_ACTPATCH", "1") == "1"

_programs: dict = {}


def _patch_act_tables():
    """Make Prelu resolve to the same act-table set as Sigmoid so the
    compiler emits a single hoisted ACT_TABLE_LOAD.  Set positions (the
    act_func_set_id namespace) are preserved; only membership shrinks."""
    from concourse.hw_specs import get_activation_tables as _orig

    T = mybir.ActivationFunctionType

    def patched(arch):
        tabs = {k: set(v) for k, v in _orig(arch).items()}
        shared = tabs.get("sigmoid_and_others")
        if not shared or T.Prelu not in shared or T.Sigmoid not in shared:
            return tabs
        for name, fns in tabs.items():
            if name != "sigmoid_and_others":
                fns.discard(T.Prelu)
                fns.discard(T.Sigmoid)
        return tabs

    bacc.get_activation_tables = patched


if ACT_TABLE_PATCH:
    _patch_act_tables()


def _build_program(use_batt: bool):
    nc = bacc.Bacc(
        "TRN2", target_bir_lowering=False, debug=False, num_devices=NCORES
    )
    dA = nc.dram_tensor("dA", [64, XA], bf16, kind="ExternalInput")
    dB = nc.dram_tensor("dB", [128, XB], bf16, kind="ExternalInput")
    out_d = nc.dram_tensor("out", [K, NO], f32, kind="ExternalOutput")
    mult, add = mybir.AluOpType.mult, mybir.AluOpType.add

    with tile.TileContext(nc) as tc, ExitStack() as ctx:
        const = ctx.enter_context(tc.tile_pool(name="const", bufs=1))
        work = ctx.enter_context(tc.tile_pool(name="work", bufs=2))
        ps_z = ctx.enter_context(tc.tile_pool(name="ps_z", bufs=2, space="PSUM"))
        ps_o = ctx.enter_context(tc.tile_pool(name="ps_o", bufs=1, space="PSUM"))

        tA = const.tile([64, XA], bf16, tag="tA", name="tA")
        nc.sync.dma_start(tA[:], dA[:])
        tB = const.tile([128, XB], bf16, tag="tB", name="tB")
        nc.scalar.dma_start(tB[:], dB[:])

        w3bd = tA[:, A_W3BD : A_W3BD + NO]
        wattc = tB[:, B_WATT : B_WATT + NO]

        zs = []
        for c in range(ECH):
            z = ps_z.tile([128, NO], f32, tag="z", name=f"z{c}")
            nc.tensor.matmul(
                z[:], tA[:, A_BONDS + c * 128 : A_BONDS + (c + 1) * 128], w3bd,
                start=True, stop=True,
            )
            zs.append(z)

        if use_batt:
            battf = work.tile([128, 1], f32, tag="battf", name="battf")
            nc.vector.tensor_copy(battf[:], tB[:, B_BATT : B_BATT + 1])

        lats = []
        for c in range(ECH):
            agc = tB[:, B_AG + c * NO : B_AG + (c + 1) * NO]
            t1 = work.tile([128, NO], bf16, tag="t1", name=f"t1_{c}")
            nc.vector.tensor_add(t1[:], zs[c][:], agc)
            pre = t1[:]
            lat = const.tile([128, NO], bf16, tag=f"lat{c}", name=f"lat{c}")
            if LEAKY_ON_ACT:
                nc.scalar.activation(
                    lat[:], pre, mybir.ActivationFunctionType.Prelu,
                    alpha=NEG_SLOPE,
                )
            else:
                tmp = work.tile([128, NO], bf16, tag="tmp", name=f"tmp{c}")
                nc.vector.tensor_scalar_mul(tmp[:], pre, NEG_SLOPE)
                nc.vector.tensor_max(lat[:], tmp[:], pre)
            lats.append(lat)

            junk = work.tile([128, NO], bf16, tag="junk", name=f"junk{c}")
            for b in range(B):
                scol = work.tile([128, 1], f32, tag=f"scol{b}", name=f"scol{c}_{b}")
                nc.vector.scalar_tensor_tensor(
                    out=junk[:, b * COUT : (b + 1) * COUT],
                    in0=lat[:, b * COUT : (b + 1) * COUT], scalar=1.0,
                    in1=wattc[:, b * COUT : (b + 1) * COUT],
                    op0=mult, op1=mult, accum_out=scol[:],
                )
                att1 = work.tile([128, 1], f32, tag=f"att{b}", name=f"att{c}_{b}")
                nc.scalar.activation(
                    att1[:], scol[:], mybir.ActivationFunctionType.Sigmoid,
                    bias=battf[:, 0:1] if use_batt else 0.0,
                )
                sl = lat[:, b * COUT : (b + 1) * COUT]
                nc.vector.tensor_scalar_mul(sl, sl, att1[:])

        # per-batch scatter accumulators -> two copies and two output DMAs
        # on the two HW rings, so the out tail overlaps instead of chaining
        o_bs = [ps_o.tile([K, COUT], f32, tag=f"ob{b}", name=f"ob{b}") for b in range(B)]
        for c in range(ECH):
            oh2c = tB[:, B_OH2 + c * K : B_OH2 + (c + 1) * K]
            for b in range(B):
                nc.tensor.matmul(
                    o_bs[b][:], oh2c,
                    lats[c][:, b * COUT : (b + 1) * COUT],
                    start=(c == 0), stop=(c == ECH - 1),
                )
        o_sb0 = work.tile([K, COUT], f32, tag="osb0", name="osb0")
        nc.vector.tensor_copy(o_sb0[:], o_bs[0][:])
        nc.sync.dma_start(out_d[:, 0:COUT], o_sb0[:])
        o_sb1 = work.tile([K, COUT], f32, tag="osb1", name="osb1")
        nc.vector.tensor_copy(o_sb1[:], o_bs[1][:])
        nc.scalar.dma_start(out_d[:, COUT:NO], o_sb1[:])

    nc.compile()
    return nc


def _get_program(use_batt: bool):
    if use_batt not in _programs:
        _programs[use_batt] = _build_program(use_batt)
    return _programs[use_batt]


def _prepare(inputs):
    """Host fold: group-mean weights, node tables through W, per-edge Ag."""
    sites1 = np.asarray(inputs["sites1"], np.float32)
    sites2 = np.asarray(inputs["sites2"], np.float32)
    bonds = np.asarray(inputs["bonds"], np.float32)
    W_eq = np.asarray(inputs["W_eq"], np.float32)
    b_eq = np.asarray(inputs["b_eq"], np.float32)
    W_att = np.asarray(inputs["W_att"], np.float32)
    b_att = np.asarray(inputs["b_att"], np.float32)
    idx1 = np.asarray(inputs["idx1"])
    idx2 = np.asarray(inputs["idx2"])

    W_eff = W_eq.mean(axis=0)                       # [F, COUT]
    A1 = sites1 @ W_eff[0:CIN]                      # [B, N1, COUT]
    A2 = sites2 @ W_eff[CIN : 2 * CIN]              # [B, K, COUT]
    W3 = W_eff[2 * CIN : F]                         # [CB, COUT]
    Ag = A1[:, idx1] + A2[:, idx2] + b_eq[None, None, :]   # [B, E, COUT]

    w3bd = np.zeros((64, NO), np.float32)
    w3bd[0:CB, 0:COUT] = W3
    w3bd[CB:64, COUT:NO] = W3

    oh2 = (idx2[:, None] == np.arange(K)[None, :]).astype(np.float32)  # [E, K]
    use_batt = bool(np.any(b_att != 0.0))

    in_maps = []
    for m in range(NCORES):
        dA = np.zeros((64, XA), np.float32)
        dB = np.zeros((128, XB), np.float32)
        for c in range(ECH):
            rows = slice(m * ES + c * 128, m * ES + (c + 1) * 128)
            for b in range(B):
                dA[b * CB : (b + 1) * CB, A_BONDS + c * 128 : A_BONDS + (c + 1) * 128] = (
                    bonds[b, rows].T
                )
                dB[:, B_AG + c * NO + b * COUT : B_AG + c * NO + (b + 1) * COUT] = (
                    Ag[b, rows]
                )
            dB[:, B_OH2 + c * K : B_OH2 + (c + 1) * K] = oh2[rows]
        dA[:, A_W3BD : A_W3BD + NO] = w3bd
        for b in range(B):
            dB[:, B_WATT + b * COUT : B_WATT + (b + 1) * COUT] = W_att[:, 0][None, :]
        dB[:, B_BATT] = b_att[0]
        in_maps.append({
            "dA": dA.astype(ml_dtypes.bfloat16),
            "dB": dB.astype(ml_dtypes.bfloat16),
        })
    return use_batt, in_maps


def _numpy_fallback(inputs):
    """Exact reference semantics (pathological inputs only)."""
    sites1 = np.asarray(inputs["sites1"], np.float32)
    sites2 = np.asarray(inputs["sites2"], np.float32)
    bonds = np.asarray(inputs["bonds"], np.float32)
    W_eq = np.asarray(inputs["W_eq"], np.float32)
    b_eq = np.asarray(inputs["b_eq"], np.float32)
    W_att = np.asarray(inputs["W_att"], np.float32)
    b_att = np.asarray(inputs["b_att"], np.float32)
    idx2_oh = np.asarray(inputs["idx2_oh"], np.float32)
    idx1 = np.asarray(inputs["idx1"])
    idx2 = np.asarray(inputs["idx2"])
    perms1 = np.asarray(inputs["perms1"])
    perms2 = np.asarray(inputs["perms2"])
    Gn, Kn = perms1.shape
    inv2 = np.argsort(perms2, axis=1)
    out = np.zeros((B, Kn, COUT), np.float32)
    for b in range(B):
        vec = np.concatenate([sites1[b][idx1], sites2[b][idx2], bonds[b]], axis=1)
        zg = np.stack([vec @ W_eq[g] for g in range(Gn)])        # [G, E, O]
        y = np.zeros((E, COUT, Kn), np.float32)
        for g in range(Gn):
            sel = idx2_oh[:, perms1[g][inv2[g]]]                 # [E, K]
            y += zg[g][:, :, None] * sel[:, None, :]
        y /= Gn
        y = y + b_eq[None, :, None]
        y = np.maximum(y, NEG_SLOPE * y)
        lat = np.einsum("eok,ek->eo", y, idx2_oh)
        att = 1.0 / (1.0 + np.exp(-(lat @ W_att[:, 0] + b_att[0])))
        lat = att[:, None] * lat
        np.add.at(out[b], idx2, lat)
    return out


def _run(inputs, trace=False, **run_kwargs):
    idx2 = np.asarray(inputs["idx2"])
    idx2_oh = np.asarray(inputs["idx2_oh"], np.float32)
    expected_oh = (idx2[:, None] == np.arange(K)[None, :]).astype(np.float32)
    perms1 = np.asarray(inputs["perms1"])
    perms2 = np.asarray(inputs["perms2"])
    inv2 = np.argsort(perms2, axis=1)
    c = np.take_along_axis(perms1, inv2, axis=1) == np.arange(K)[None, :]
    if not (np.array_equal(idx2_oh, expected_oh) and c.all()):
        return _numpy_fallback(inputs), None

    use_batt, in_maps = _prepare(inputs)
    nc = _get_program(use_batt)
    res = None
    last_err = None
    for _attempt in range(3):
        try:
            res = run_bass_kernel_spmd(
                nc, in_maps, list(range(NCORES)), trace=trace, **run_kwargs
            )
            break
        except Exception as e:  # transient device/tunnel flakes
            last_err = e
    if res is None:
        raise last_err
    acc = np.zeros((K, NO), np.float32)
    for r in res.results:
        acc += r["out"]
    out = acc.reshape(K, B, COUT).transpose(1, 0, 2)
    return np.ascontiguousarray(out), res


def kernel(**inputs) -> np.ndarray:
    out, _ = _run(inputs)
    return out
